# revision 1
# baseline (speedup 1.0000x reference)
"""FMoELinear grouped GEMM on 8 Trainium2 NeuronCores (expert parallelism).

Strategy
--------
Tokens arrive pre-grouped contiguously by expert, and the per-expert counts
are host-visible routing metadata.  All routing therefore happens on the
host: the 64 expert segments are split into 8*G near-equal-sized "slots"
(pieces of experts, padded up to a whole number of 128-token tiles), dealt
onto the 8 cores rank-matched so that slot g has the same tile count K[g]
on every core.  That makes one SPMD Bass program valid for all 8 cores
while keeping padding ~3-5% and per-core weight traffic at ~G matrices.

Per core the device kernel computes, slot by slot:
    out[tile] = sum_k xT[k,tile].T @ wT[k] + bias      (psum accumulation)
with x shipped host-transposed ([128 in-feat partitions, token columns]) so
the PE needs no on-chip transposes, and the output written in
[128, tiles, 512] layout (partition = token-within-tile) which the host
untransposes during the gather.

Numerics: MODE selects the matmul path.
  "f32"   - native fp32 matmuls (4 cycles/row on PE)
  "f32r"  - hardware round-to-~13-bit fast fp32 (1 cycle/row), ~1e-4 rel err
  "bf16x3"- split x,w into bf16 hi+lo, 3 bf16 matmuls: xh@wh + xh@wl + xl@wh
            (~4e-6 rel err, 3 cycles/row, same DMA bytes as fp32)
"""
import sys
sys.path.insert(0, "/opt/trn_rl_repo")

import numpy as np
import ml_dtypes

import concourse.bass as bass
import concourse.tile as tile
from concourse import bacc, mybir
from concourse.bass_utils import run_bass_kernel_spmd

# If the environment requests NTFF tracing (BASS_TRACE=1) but this image's
# antenv lacks the axon profiling hook module, run_bass_kernel_spmd would
# crash on import. Register a null hook so it degrades to trace-skipped.
try:
    from antenv.axon_hooks import get_axon_ntff_profile_hook as _hook_probe  # noqa: F401
except ImportError:
    import types as _types

    import antenv as _antenv

    _mod = _types.ModuleType("antenv.axon_hooks")
    _mod.get_axon_ntff_profile_hook = lambda: None
    _mod.set_axon_ntff_profile_hook = lambda h: None
    sys.modules.setdefault("antenv.axon_hooks", _mod)
    _antenv.axon_hooks = sys.modules["antenv.axon_hooks"]

F32 = mybir.dt.float32
F32R = mybir.dt.float32r
BF16 = mybir.dt.bfloat16

NUM_EXPERT = 64
IN_FEAT = 512
OUT_FEAT = 512
P = 128
KT = IN_FEAT // P          # 4 contraction k-tiles
NCORES = 8
G_SLOTS = 10               # slots per core
CX = 12                    # token tiles per x-DMA chunk
XBUFS = 4
OBUFS = 2
WBUFS = 2

MODE = "f32r"              # "f32" | "f32r" | "bf16x3"
TRACE = False              # set True (e.g. from test.py) to profile
LAST_RESULT = None         # BassKernelResults of the last run

_program_cache = {}


# ----------------------------------------------------------------- schedule
def _schedule(counts):
    """Split experts into 8*G pieces, deal rank-octets onto cores.

    Returns (K, slots): K[g] = tile count of slot g (same on all cores);
    slots[core][g] = (expert, row_start, nrows) with nrows <= K[g]*128.
    """
    counts = [int(c) for c in counts]
    starts = np.concatenate([[0], np.cumsum(counts)]).astype(np.int64)
    tiles = [(c + P - 1) // P for c in counts]
    nt = sum(tiles)
    M = NCORES * G_SLOTS

    live = [e for e in range(NUM_EXPERT) if tiles[e] > 0]
    ideal = max(nt / M, 1e-9)
    p = {e: max(1, round(tiles[e] / ideal)) for e in live}
    # adjust piece count to exactly M (or len(live) if M smaller)
    want = max(M, len(live))
    while sum(p.values()) > want:
        cand = [e for e in live if p[e] > 1]
        if not cand:
            break
        e = min(cand, key=lambda e: tiles[e] / p[e])
        p[e] -= 1
    while sum(p.values()) < want:
        e = max(live, key=lambda e: tiles[e] / p[e])
        p[e] += 1

    pieces = []  # (ntiles, expert, tile_lo, tile_hi)
    for e in live:
        n = p[e]
        base, rem = divmod(tiles[e], n)
        lo = 0
        for i in range(n):
            sz = base + (1 if i < rem else 0)
            if sz == 0:
                continue
            pieces.append((sz, e, lo, lo + sz))
            lo += sz
    # pad with empty pieces so count is a multiple of NCORES
    G = (len(pieces) + NCORES - 1) // NCORES
    while len(pieces) < NCORES * G:
        pieces.append((0, 0, 0, 0))
    pieces.sort(key=lambda t: -t[0])

    K = []
    slots = [[] for _ in range(NCORES)]
    for g in range(G):
        octet = pieces[g * NCORES:(g + 1) * NCORES]
        K.append(octet[0][0])
        for i, (sz, e, tlo, thi) in enumerate(octet):
            r0 = starts[e] + tlo * P
            r1 = min(starts[e] + thi * P, starts[e] + counts[e])
            slots[i].append((e, int(r0), max(0, int(r1 - r0))))
    # drop trailing zero-size slots
    while K and K[-1] == 0:
        K.pop()
        for s in slots:
            s.pop()
    return K, slots


# ------------------------------------------------------------ device program
def _build_program(K, mode):
    G = len(K)
    T = sum(K)
    nc = bacc.Bacc(None)

    if mode == "bf16x3":
        xh_d = nc.declare_dram_parameter("xh", [P, KT, T * P], BF16, isOutput=False)
        xl_d = nc.declare_dram_parameter("xl", [P, KT, T * P], BF16, isOutput=False)
        wh_d = nc.declare_dram_parameter("wh", [G, P, KT, OUT_FEAT], BF16, isOutput=False)
        wl_d = nc.declare_dram_parameter("wl", [G, P, KT, OUT_FEAT], BF16, isOutput=False)
    else:
        mmdt = F32R if mode == "f32r" else F32
        xt_d = nc.declare_dram_parameter("xt", [P, KT, T * P], mmdt, isOutput=False)
        wt_d = nc.declare_dram_parameter("wt", [G, P, KT, OUT_FEAT], mmdt, isOutput=False)
    b_d = nc.declare_dram_parameter("bias", [G, 1, OUT_FEAT], F32, isOutput=False)
    out_d = nc.declare_dram_parameter("out", [P, T, OUT_FEAT], F32, isOutput=True)

    with tile.TileContext(nc) as tc:
        with (
            tc.tile_pool(name="w", bufs=WBUFS) as wp,
            tc.tile_pool(name="x", bufs=XBUFS) as xp,
            tc.tile_pool(name="b", bufs=2) as bp,
            tc.tile_pool(name="o", bufs=OBUFS) as op,
            tc.tile_pool(name="ps", bufs=8, space=bass.MemorySpace.PSUM) as pp,
        ):
            off = 0
            for g in range(G):
                kg = K[g]
                if mode == "bf16x3":
                    wh_sb = wp.tile([P, KT, OUT_FEAT], BF16, tag="wh")
                    nc.gpsimd.dma_start(wh_sb[:], wh_d[g])
                    wl_sb = wp.tile([P, KT, OUT_FEAT], BF16, tag="wl")
                    nc.gpsimd.dma_start(wl_sb[:], wl_d[g])
                else:
                    w_sb = wp.tile([P, KT, OUT_FEAT], mmdt, tag="w")
                    nc.gpsimd.dma_start(w_sb[:], wt_d[g])
                b1_sb = bp.tile([1, OUT_FEAT], F32, tag="b1")
                nc.gpsimd.dma_start(b1_sb[:], b_d[g])
                b_sb = bp.tile([P, OUT_FEAT], F32, tag="b")
                nc.gpsimd.partition_broadcast(b_sb[:], b1_sb[:])

                for c0 in range(0, kg, CX):
                    cw = min(CX, kg - c0)
                    w_cols = cw * P
                    col0 = (off + c0) * P
                    if mode == "bf16x3":
                        xh_sb = xp.tile([P, KT, CX * P], BF16, tag="xh")
                        nc.sync.dma_start(
                            xh_sb[:, :, :w_cols], xh_d[:, :, col0:col0 + w_cols])
                        xl_sb = xp.tile([P, KT, CX * P], BF16, tag="xl")
                        nc.sync.dma_start(
                            xl_sb[:, :, :w_cols], xl_d[:, :, col0:col0 + w_cols])
                    else:
                        x_sb = xp.tile([P, KT, CX * P], mmdt, tag="x")
                        nc.sync.dma_start(
                            x_sb[:, :, :w_cols], xt_d[:, :, col0:col0 + w_cols])
                    o_sb = op.tile([P, CX, OUT_FEAT], F32)
                    for t in range(cw):
                        ps = pp.tile([P, OUT_FEAT], F32)
                        if mode == "bf16x3":
                            terms = []
                            for k in range(KT):
                                sl = slice(t * P, (t + 1) * P)
                                terms.append((xh_sb[:, k, sl], wh_sb[:, k, :]))
                                terms.append((xh_sb[:, k, sl], wl_sb[:, k, :]))
                                terms.append((xl_sb[:, k, sl], wh_sb[:, k, :]))
                        else:
                            terms = [
                                (x_sb[:, k, t * P:(t + 1) * P], w_sb[:, k, :])
                                for k in range(KT)
                            ]
                        for i, (lhsT, rhs) in enumerate(terms):
                            nc.tensor.matmul(
                                ps[:], lhsT, rhs,
                                start=(i == 0), stop=(i == len(terms) - 1))
                        nc.vector.tensor_add(o_sb[:, t, :], ps[:], b_sb[:])
                    nc.scalar.dma_start(
                        out_d[:, off + c0:off + c0 + cw, :], o_sb[:, :cw, :])
                off += kg
    nc.compile()
    return nc


# ------------------------------------------------------------------- kernel
def _bf16_split(a):
    hi = a.astype(ml_dtypes.bfloat16)
    lo = (a - hi.astype(np.float32)).astype(ml_dtypes.bfloat16)
    return hi, lo


def kernel(inp, fwd_expert_count, weight, bias):
    inp = np.asarray(inp, dtype=np.float32)
    weight = np.asarray(weight, dtype=np.float32)
    bias = np.asarray(bias, dtype=np.float32)
    counts = np.asarray(fwd_expert_count)

    K, slots = _schedule(counts)
    G, T = len(K), sum(K)
    off = np.concatenate([[0], np.cumsum(K)]).astype(np.int64)

    key = (tuple(K), MODE)
    if key not in _program_cache:
        _program_cache[key] = _build_program(K, MODE)
    nc = _program_cache[key]

    # per-expert transposed weights [P, KT, OUT]: wT[p, k, o] = weight[e][o, 128k+p]
    wT = {}
    for e in set(e for s in slots for (e, _, n) in s if n > 0):
        wT[e] = np.ascontiguousarray(
            weight[e].T.reshape(KT, P, OUT_FEAT).transpose(1, 0, 2))

    in_maps = []
    for core in range(NCORES):
        if MODE == "bf16x3":
            xh = np.zeros((P, KT, T * P), dtype=ml_dtypes.bfloat16)
            xl = np.zeros((P, KT, T * P), dtype=ml_dtypes.bfloat16)
        else:
            xt = np.zeros((P, KT, T * P), dtype=np.float32)
        if MODE == "bf16x3":
            wh = np.zeros((G, P, KT, OUT_FEAT), dtype=ml_dtypes.bfloat16)
            wl = np.zeros((G, P, KT, OUT_FEAT), dtype=ml_dtypes.bfloat16)
        else:
            wt = np.zeros((G, P, KT, OUT_FEAT), dtype=np.float32)
        brep = np.zeros((G, 1, OUT_FEAT), dtype=np.float32)

        for g, (e, r0, n) in enumerate(slots[core]):
            if n > 0:
                blk = inp[r0:r0 + n].T.reshape(KT, P, n).transpose(1, 0, 2)
                col0 = int(off[g]) * P
                if MODE == "bf16x3":
                    h, l = _bf16_split(blk)
                    xh[:, :, col0:col0 + n] = h
                    xl[:, :, col0:col0 + n] = l
                else:
                    xt[:, :, col0:col0 + n] = blk
                if MODE == "bf16x3":
                    h, l = _bf16_split(wT[e])
                    wh[g] = h
                    wl[g] = l
                else:
                    wt[g] = wT[e]
                brep[g, 0] = bias[e]
        if MODE == "bf16x3":
            in_maps.append({"xh": xh, "xl": xl, "wh": wh, "wl": wl, "bias": brep})
        else:
            in_maps.append({"xt": xt, "wt": wt, "bias": brep})

    global LAST_RESULT
    res = run_bass_kernel_spmd(
        nc, in_maps, list(range(NCORES)),
        trace=TRACE, trace_cores=list(range(NCORES)) if TRACE else None,
        stitch_traces=False)
    LAST_RESULT = res

    out = np.empty((int(np.sum(np.asarray(counts, dtype=np.int64))), OUT_FEAT),
                   dtype=np.float32)
    for core in range(NCORES):
        arr = res.results[core]["out"]  # [P, T, OUT]
        for g, (e, r0, n) in enumerate(slots[core]):
            if n > 0:
                o0 = int(off[g])
                kg = K[g]
                blk = arr[:, o0:o0 + kg, :].transpose(1, 0, 2).reshape(kg * P, OUT_FEAT)
                out[r0:r0 + n] = blk[:n]
    return out



# revision 5
# speedup vs baseline: 1.4811x; 1.4811x over previous
"""FMoELinear grouped GEMM on 8 Trainium2 NeuronCores (expert parallelism).

Strategy
--------
Tokens arrive pre-grouped contiguously by expert, and the per-expert counts
are host-visible routing metadata.  All routing therefore happens on the
host: the 64 expert segments are split into 8*G near-equal-sized "slots"
(pieces of experts, padded up to a whole number of 128-token tiles), dealt
onto the 8 cores rank-matched so that slot g has the same tile count K[g]
on every core.  That makes one SPMD Bass program valid for all 8 cores
while keeping padding ~3-5% and per-core weight traffic at ~G matrices.

Per core the device kernel computes, slot by slot:
    out[tile] = sum_k xT[k,tile].T @ wT[k] + bias      (psum accumulation)
with x shipped host-transposed ([128 in-feat partitions, token columns]) so
the PE needs no on-chip transposes, and the output written in
[128, tiles, 512] layout (partition = token-within-tile) which the host
untransposes during the gather.

Numerics: MODE selects the matmul path.
  "f32"   - native fp32 matmuls (4 cycles/row on PE)
  "f32r"  - hardware round-to-~13-bit fast fp32 (1 cycle/row), ~1e-4 rel err
  "bf16x3"- split x,w into bf16 hi+lo, 3 bf16 matmuls: xh@wh + xh@wl + xl@wh
            (~4e-6 rel err, 3 cycles/row, same DMA bytes as fp32)
"""
import sys
sys.path.insert(0, "/opt/trn_rl_repo")

import numpy as np
import ml_dtypes

import concourse.bass as bass
import concourse.tile as tile
from concourse import bacc, mybir
from concourse.bass_utils import run_bass_kernel_spmd

# If the environment requests NTFF tracing (BASS_TRACE=1) but this image's
# antenv lacks the axon profiling hook module, run_bass_kernel_spmd would
# crash on import. Register a null hook so it degrades to trace-skipped.
try:
    from antenv.axon_hooks import get_axon_ntff_profile_hook as _hook_probe  # noqa: F401
except ImportError:
    import types as _types

    import antenv as _antenv

    _mod = _types.ModuleType("antenv.axon_hooks")
    _mod.get_axon_ntff_profile_hook = lambda: None
    _mod.set_axon_ntff_profile_hook = lambda h: None
    sys.modules.setdefault("antenv.axon_hooks", _mod)
    _antenv.axon_hooks = sys.modules["antenv.axon_hooks"]

F32 = mybir.dt.float32
F32R = mybir.dt.float32r
BF16 = mybir.dt.bfloat16
F16 = mybir.dt.float16

NUM_EXPERT = 64
IN_FEAT = 512
OUT_FEAT = 512
P = 128
KT = IN_FEAT // P          # 4 contraction k-tiles
NCORES = 8
G_SLOTS = 10               # slots per core
CX = 12                    # token tiles per x-DMA chunk
XBUFS = 4
OBUFS = 2
WBUFS = 2

MODE = "f32r"              # "f32" | "f32r" | "bf16x3"
TRACE = False              # set True (e.g. from test.py) to profile
LAST_RESULT = None         # BassKernelResults of the last run

_program_cache = {}


# ----------------------------------------------------------------- schedule
def _schedule(counts):
    """Split experts into 8*G pieces, deal rank-octets onto cores.

    Returns (K, slots): K[g] = tile count of slot g (same on all cores);
    slots[core][g] = (expert, row_start, nrows) with nrows <= K[g]*128.
    """
    counts = [int(c) for c in counts]
    starts = np.concatenate([[0], np.cumsum(counts)]).astype(np.int64)
    tiles = [(c + P - 1) // P for c in counts]
    nt = sum(tiles)
    M = NCORES * G_SLOTS

    live = [e for e in range(NUM_EXPERT) if tiles[e] > 0]
    ideal = max(nt / M, 1e-9)
    p = {e: max(1, round(tiles[e] / ideal)) for e in live}
    # adjust piece count to exactly M (or len(live) if M smaller)
    want = max(M, len(live))
    while sum(p.values()) > want:
        cand = [e for e in live if p[e] > 1]
        if not cand:
            break
        e = min(cand, key=lambda e: tiles[e] / p[e])
        p[e] -= 1
    while sum(p.values()) < want:
        e = max(live, key=lambda e: tiles[e] / p[e])
        p[e] += 1

    pieces = []  # (ntiles, expert, tile_lo, tile_hi)
    for e in live:
        n = p[e]
        base, rem = divmod(tiles[e], n)
        lo = 0
        for i in range(n):
            sz = base + (1 if i < rem else 0)
            if sz == 0:
                continue
            pieces.append((sz, e, lo, lo + sz))
            lo += sz
    # pad with empty pieces so count is a multiple of NCORES
    G = (len(pieces) + NCORES - 1) // NCORES
    while len(pieces) < NCORES * G:
        pieces.append((0, 0, 0, 0))
    pieces.sort(key=lambda t: -t[0])

    K = []
    slots = [[] for _ in range(NCORES)]
    for g in range(G):
        octet = pieces[g * NCORES:(g + 1) * NCORES]
        K.append(octet[0][0])
        for i, (sz, e, tlo, thi) in enumerate(octet):
            r0 = starts[e] + tlo * P
            r1 = min(starts[e] + thi * P, starts[e] + counts[e])
            slots[i].append((e, int(r0), max(0, int(r1 - r0))))
    # drop trailing zero-size slots
    while K and K[-1] == 0:
        K.pop()
        for s in slots:
            s.pop()
    return K, slots


# ------------------------------------------------------------ device program
def _build_program(K, mode):
    G = len(K)
    T = sum(K)
    nc = bacc.Bacc(None)

    if mode == "bf16x3":
        xh_d = nc.declare_dram_parameter("xh", [P, KT, T * P], BF16, isOutput=False)
        xl_d = nc.declare_dram_parameter("xl", [P, KT, T * P], BF16, isOutput=False)
        wh_d = nc.declare_dram_parameter("wh", [G, P, KT, OUT_FEAT], BF16, isOutput=False)
        wl_d = nc.declare_dram_parameter("wl", [G, P, KT, OUT_FEAT], BF16, isOutput=False)
    else:
        mmdt = {"f32r": F32R, "f32": F32, "bf16": BF16}[mode]
        xt_d = nc.declare_dram_parameter("xt", [P, KT, T * P], mmdt, isOutput=False)
        wt_d = nc.declare_dram_parameter("wt", [G, P, KT, OUT_FEAT], mmdt, isOutput=False)
    outdt = F16 if mode == "bf16" else F32
    b_d = nc.declare_dram_parameter("bias", [G, 1, OUT_FEAT], F32, isOutput=False)
    out_d = nc.declare_dram_parameter("out", [P, T, OUT_FEAT], outdt, isOutput=True)

    with tile.TileContext(nc) as tc:
        with (
            tc.tile_pool(name="w", bufs=WBUFS) as wp,
            tc.tile_pool(name="x", bufs=XBUFS) as xp,
            tc.tile_pool(name="b", bufs=2) as bp,
            tc.tile_pool(name="o", bufs=OBUFS) as op,
            tc.tile_pool(name="ps", bufs=8, space=bass.MemorySpace.PSUM) as pp,
        ):
            off = 0
            for g in range(G):
                kg = K[g]
                if mode == "bf16x3":
                    wh_sb = wp.tile([P, KT, OUT_FEAT], BF16, tag="wh")
                    nc.gpsimd.dma_start(wh_sb[:], wh_d[g])
                    wl_sb = wp.tile([P, KT, OUT_FEAT], BF16, tag="wl")
                    nc.gpsimd.dma_start(wl_sb[:], wl_d[g])
                else:
                    w_sb = wp.tile([P, KT, OUT_FEAT], mmdt, tag="w")
                    nc.gpsimd.dma_start(w_sb[:], wt_d[g])
                b1_sb = bp.tile([1, OUT_FEAT], F32, tag="b1")
                nc.gpsimd.dma_start(b1_sb[:], b_d[g])
                b_sb = bp.tile([P, OUT_FEAT], F32, tag="b")
                nc.gpsimd.partition_broadcast(b_sb[:], b1_sb[:])

                for c0 in range(0, kg, CX):
                    cw = min(CX, kg - c0)
                    w_cols = cw * P
                    col0 = (off + c0) * P
                    if mode == "bf16x3":
                        xh_sb = xp.tile([P, KT, CX * P], BF16, tag="xh")
                        nc.sync.dma_start(
                            xh_sb[:, :, :w_cols], xh_d[:, :, col0:col0 + w_cols])
                        xl_sb = xp.tile([P, KT, CX * P], BF16, tag="xl")
                        nc.sync.dma_start(
                            xl_sb[:, :, :w_cols], xl_d[:, :, col0:col0 + w_cols])
                    else:
                        x_sb = xp.tile([P, KT, CX * P], mmdt, tag="x")
                        nc.sync.dma_start(
                            x_sb[:, :, :w_cols], xt_d[:, :, col0:col0 + w_cols])
                    o_sb = op.tile([P, CX, OUT_FEAT], outdt)
                    for t in range(cw):
                        ps = pp.tile([P, OUT_FEAT], F32)
                        if mode == "bf16x3":
                            terms = []
                            for k in range(KT):
                                sl = slice(t * P, (t + 1) * P)
                                terms.append((xh_sb[:, k, sl], wh_sb[:, k, :]))
                                terms.append((xh_sb[:, k, sl], wl_sb[:, k, :]))
                                terms.append((xl_sb[:, k, sl], wh_sb[:, k, :]))
                        else:
                            terms = [
                                (x_sb[:, k, t * P:(t + 1) * P], w_sb[:, k, :])
                                for k in range(KT)
                            ]
                        for i, (lhsT, rhs) in enumerate(terms):
                            nc.tensor.matmul(
                                ps[:], lhsT, rhs,
                                start=(i == 0), stop=(i == len(terms) - 1))
                        nc.vector.tensor_add(o_sb[:, t, :], ps[:], b_sb[:])
                    nc.scalar.dma_start(
                        out_d[:, off + c0:off + c0 + cw, :], o_sb[:, :cw, :])
                off += kg
    nc.compile()
    return nc


# ------------------------------------------------------------------- kernel
def _bf16_split(a):
    hi = a.astype(ml_dtypes.bfloat16)
    lo = (a - hi.astype(np.float32)).astype(ml_dtypes.bfloat16)
    return hi, lo


def kernel(inp, fwd_expert_count, weight, bias):
    inp = np.asarray(inp, dtype=np.float32)
    weight = np.asarray(weight, dtype=np.float32)
    bias = np.asarray(bias, dtype=np.float32)
    counts = np.asarray(fwd_expert_count)

    K, slots = _schedule(counts)
    G, T = len(K), sum(K)
    off = np.concatenate([[0], np.cumsum(K)]).astype(np.int64)

    key = (tuple(K), MODE)
    if key not in _program_cache:
        _program_cache[key] = _build_program(K, MODE)
    nc = _program_cache[key]

    # per-expert transposed weights [P, KT, OUT]: wT[p, k, o] = weight[e][o, 128k+p]
    wT = {}
    for e in set(e for s in slots for (e, _, n) in s if n > 0):
        wT[e] = np.ascontiguousarray(
            weight[e].T.reshape(KT, P, OUT_FEAT).transpose(1, 0, 2))

    in_maps = []
    for core in range(NCORES):
        npdt = ml_dtypes.bfloat16 if MODE == "bf16" else np.float32
        if MODE == "bf16x3":
            xh = np.zeros((P, KT, T * P), dtype=ml_dtypes.bfloat16)
            xl = np.zeros((P, KT, T * P), dtype=ml_dtypes.bfloat16)
        else:
            xt = np.zeros((P, KT, T * P), dtype=npdt)
        if MODE == "bf16x3":
            wh = np.zeros((G, P, KT, OUT_FEAT), dtype=ml_dtypes.bfloat16)
            wl = np.zeros((G, P, KT, OUT_FEAT), dtype=ml_dtypes.bfloat16)
        else:
            wt = np.zeros((G, P, KT, OUT_FEAT), dtype=npdt)
        brep = np.zeros((G, 1, OUT_FEAT), dtype=np.float32)

        for g, (e, r0, n) in enumerate(slots[core]):
            if n > 0:
                blk = inp[r0:r0 + n].T.reshape(KT, P, n).transpose(1, 0, 2)
                col0 = int(off[g]) * P
                if MODE == "bf16x3":
                    h, l = _bf16_split(blk)
                    xh[:, :, col0:col0 + n] = h
                    xl[:, :, col0:col0 + n] = l
                else:
                    xt[:, :, col0:col0 + n] = blk
                if MODE == "bf16x3":
                    h, l = _bf16_split(wT[e])
                    wh[g] = h
                    wl[g] = l
                else:
                    wt[g] = wT[e]
                brep[g, 0] = bias[e]
        if MODE == "bf16x3":
            in_maps.append({"xh": xh, "xl": xl, "wh": wh, "wl": wl, "bias": brep})
        else:
            in_maps.append({"xt": xt, "wt": wt, "bias": brep})

    global LAST_RESULT
    res = run_bass_kernel_spmd(
        nc, in_maps, list(range(NCORES)),
        trace=TRACE, trace_cores=list(range(NCORES)) if TRACE else None,
        stitch_traces=False)
    LAST_RESULT = res

    out = np.empty((int(np.sum(np.asarray(counts, dtype=np.int64))), OUT_FEAT),
                   dtype=np.float32)
    for core in range(NCORES):
        arr = res.results[core]["out"]  # [P, T, OUT]
        for g, (e, r0, n) in enumerate(slots[core]):
            if n > 0:
                o0 = int(off[g])
                kg = K[g]
                blk = arr[:, o0:o0 + kg, :].transpose(1, 0, 2).reshape(kg * P, OUT_FEAT)
                out[r0:r0 + n] = blk[:n]
    return out



# revision 6
# speedup vs baseline: 1.8594x; 1.2554x over previous
"""FMoELinear grouped GEMM on 8 Trainium2 NeuronCores (expert parallelism).

Strategy
--------
Tokens arrive pre-grouped contiguously by expert, and the per-expert counts
are host-visible routing metadata.  All routing therefore happens on the
host: the 64 expert segments are split into 8*G near-equal "pieces"
(arbitrary token offsets, optimized by local search so that the 8 pieces
sharing a slot rank have near-equal tile counts), dealt onto the 8 cores
rank-matched so that slot g has the same tile count K[g] on every core.
That makes one SPMD Bass program valid for all 8 cores with ~4% padding.

Per core the device kernel computes, slot by slot:
    out[tile] = sum_k xT[k,tile].T @ wT[k] + bias      (psum accumulation)
with x shipped host-transposed ([128 in-feat partitions, token columns]) so
the PE needs no on-chip transposes, and the output written in
[128, tiles, 512] layout (partition = token-within-tile) which the host
untransposes during the gather.

Numerics: MODE selects the matmul path.
  "bf16"  - bf16 matmuls, fp16 output (~2e-3 rel err, halves DMA bytes)
  "f32"   - native fp32 matmuls (4 cycles/row on PE)
  "f32r"  - hardware round-to-~13-bit fast fp32 (1 cycle/row), ~1e-4 rel err
"""
import sys
sys.path.insert(0, "/opt/trn_rl_repo")

import numpy as np
import ml_dtypes

import concourse.bass as bass
import concourse.tile as tile
from concourse import bacc, mybir
from concourse.bass_utils import run_bass_kernel_spmd

# If the environment requests NTFF tracing (BASS_TRACE=1) but this image's
# antenv lacks the axon profiling hook module, run_bass_kernel_spmd would
# crash on import. Register a null hook so it degrades to trace-skipped.
try:
    from antenv.axon_hooks import get_axon_ntff_profile_hook as _hook_probe  # noqa: F401
except ImportError:
    import types as _types

    import antenv as _antenv

    _mod = _types.ModuleType("antenv.axon_hooks")
    _mod.get_axon_ntff_profile_hook = lambda: None
    _mod.set_axon_ntff_profile_hook = lambda h: None
    sys.modules.setdefault("antenv.axon_hooks", _mod)
    _antenv.axon_hooks = sys.modules["antenv.axon_hooks"]

F32 = mybir.dt.float32
F32R = mybir.dt.float32r
BF16 = mybir.dt.bfloat16
F16 = mybir.dt.float16

NUM_EXPERT = 64
IN_FEAT = 512
OUT_FEAT = 512
P = 128
KT = IN_FEAT // P          # 4 contraction k-tiles
NCORES = 8
G_SLOTS = 11               # slots (pieces) per core
CX = 12                    # token tiles per x-DMA chunk
XBUFS = 6
OBUFS = 4
WBUFS = 3

MODE = "bf16"              # "bf16" | "f32" | "f32r"
TRACE = False              # set True (e.g. from test.py) to profile
LAST_RESULT = None         # BassKernelResults of the last run

_program_cache = {}
_sched_cache = {}


# ----------------------------------------------------------------- schedule
def _schedule(counts):
    """Split experts into 8*G pieces (arbitrary offsets), octets rank-matched.

    Returns (K, slots): K[g] = tile count of slot g (same on all cores);
    slots[core][g] = (expert, row_start, nrows) with nrows <= K[g]*128.
    """
    key = tuple(int(c) for c in counts)
    if key in _sched_cache:
        return _sched_cache[key]
    import heapq

    counts = [int(c) for c in counts]
    starts = np.concatenate([[0], np.cumsum(counts)]).astype(np.int64)
    live = [e for e in range(len(counts)) if counts[e] > 0]
    M = NCORES * G_SLOTS

    # LPT: split each expert into near-equal pieces, largest first
    p = {e: 1 for e in live}
    h = [(-counts[e], e) for e in live]
    heapq.heapify(h)
    while sum(p.values()) < min(M, sum(counts[e] for e in live)):
        sz, e = heapq.heappop(h)
        if p[e] >= counts[e]:
            continue
        p[e] += 1
        heapq.heappush(h, (-counts[e] / p[e], e))
    pieces = []  # [size, expert]
    for e in live:
        n = p[e]
        base, rem = divmod(counts[e], n)
        for i in range(n):
            pieces.append([base + (1 if i < rem else 0), e])
    while len(pieces) < M:
        pieces.append([0, -1])

    def T_of(ps):
        s = sorted(ps, key=lambda t: -t[0])
        return sum((s[g * NCORES][0] + P - 1) // P for g in range(G_SLOTS))

    # local search: shift tokens between same-expert sibling pieces
    rng = np.random.default_rng(1)
    best = T_of(pieces)
    sib_idx = {}
    for j, (_, e) in enumerate(pieces):
        sib_idx.setdefault(e, []).append(j)
    for _ in range(30000):
        i = int(rng.integers(len(pieces)))
        e = pieces[i][1]
        if e < 0 or len(sib_idx[e]) < 2:
            continue
        sibs = sib_idx[e]
        j = sibs[int(rng.integers(len(sibs)))]
        if j == i:
            continue
        amt = int(rng.integers(1, 257))
        if pieces[i][0] <= amt:
            continue
        pieces[i][0] -= amt
        pieces[j][0] += amt
        c = T_of(pieces)
        if c <= best:
            best = c
        else:
            pieces[i][0] += amt
            pieces[j][0] -= amt

    # assign offsets within each expert in piece order
    off_in_e = {e: 0 for e in live}
    recs = []  # (size, expert, row_start)
    for sz, e in pieces:
        if e < 0 or sz == 0:
            recs.append((0, 0, 0))
        else:
            recs.append((sz, e, int(starts[e]) + off_in_e[e]))
            off_in_e[e] += sz
    recs.sort(key=lambda t: -t[0])

    K = []
    slots = [[] for _ in range(NCORES)]
    for g in range(G_SLOTS):
        octet = recs[g * NCORES:(g + 1) * NCORES]
        K.append((octet[0][0] + P - 1) // P)
        for i, (sz, e, r0) in enumerate(octet):
            slots[i].append((e, int(r0), int(sz)))
    while K and K[-1] == 0:
        K.pop()
        for s in slots:
            s.pop()
    _sched_cache[key] = (K, slots)
    return K, slots


def _chunks(kg, first_slot):
    """Chunk sizes for a slot of kg tiles. Slot 0 ramps up (small first
    chunk so the first matmul starts early)."""
    sizes = []
    rem = kg
    if first_slot:
        for c in (4, 8):
            if rem <= 0:
                break
            c = min(c, rem)
            sizes.append(c)
            rem -= c
    while rem > 0:
        c = min(CX, rem)
        sizes.append(c)
        rem -= c
    return sizes


# ------------------------------------------------------------ device program
def _build_program(K, mode):
    G = len(K)
    T = sum(K)
    nc = bacc.Bacc(None)

    mmdt = {"f32r": F32R, "f32": F32, "bf16": BF16}[mode]
    outdt = F16 if mode == "bf16" else F32
    xt_d = nc.declare_dram_parameter("xt", [P, KT, T * P], mmdt, isOutput=False)
    wt_d = nc.declare_dram_parameter("wt", [G, P, KT, OUT_FEAT], mmdt, isOutput=False)
    b_d = nc.declare_dram_parameter("bias", [G, 1, OUT_FEAT], F32, isOutput=False)
    out_d = nc.declare_dram_parameter("out", [P, T, OUT_FEAT], outdt, isOutput=True)

    with tile.TileContext(nc) as tc:
        with (
            tc.tile_pool(name="w", bufs=WBUFS) as wp,
            tc.tile_pool(name="x", bufs=XBUFS) as xp,
            tc.tile_pool(name="b", bufs=2) as bp,
            tc.tile_pool(name="o", bufs=OBUFS) as op,
            tc.tile_pool(name="ps", bufs=8, space=bass.MemorySpace.PSUM) as pp,
        ):
            off = 0
            for g in range(G):
                kg = K[g]
                # slot 0's weights/bias go on the sync queue ahead of the x
                # chunks so the first matmul isn't blocked on a late w load
                weng = nc.sync if g == 0 else nc.gpsimd
                w_sb = wp.tile([P, KT, OUT_FEAT], mmdt, tag="w")
                weng.dma_start(w_sb[:], wt_d[g])
                b1_sb = bp.tile([1, OUT_FEAT], F32, tag="b1")
                weng.dma_start(b1_sb[:], b_d[g])
                b_sb = bp.tile([P, OUT_FEAT], F32, tag="b")
                nc.gpsimd.partition_broadcast(b_sb[:], b1_sb[:])

                c0 = 0
                for cw in _chunks(kg, g == 0):
                    w_cols = cw * P
                    col0 = (off + c0) * P
                    x_sb = xp.tile([P, KT, CX * P], mmdt, tag="x")
                    nc.sync.dma_start(
                        x_sb[:, :, :w_cols], xt_d[:, :, col0:col0 + w_cols])
                    o_sb = op.tile([P, CX, OUT_FEAT], outdt)
                    h = (cw + 1) // 2  # store split point
                    for t in range(cw):
                        ps = pp.tile([P, OUT_FEAT], F32)
                        for k in range(KT):
                            nc.tensor.matmul(
                                ps[:], x_sb[:, k, t * P:(t + 1) * P],
                                w_sb[:, k, :],
                                start=(k == 0), stop=(k == KT - 1))
                        nc.vector.tensor_add(o_sb[:, t, :], ps[:], b_sb[:])
                        # store each half as soon as its adds are done
                        if t == h - 1:
                            nc.scalar.dma_start(
                                out_d[:, off + c0:off + c0 + h, :],
                                o_sb[:, :h, :])
                        elif t == cw - 1 and cw > h:
                            nc.scalar.dma_start(
                                out_d[:, off + c0 + h:off + c0 + cw, :],
                                o_sb[:, h:cw, :])
                    c0 += cw
                off += kg
    nc.compile()
    return nc


# ------------------------------------------------------------------- kernel
def kernel(inp, fwd_expert_count, weight, bias):
    inp = np.asarray(inp, dtype=np.float32)
    weight = np.asarray(weight, dtype=np.float32)
    bias = np.asarray(bias, dtype=np.float32)
    counts = np.asarray(fwd_expert_count)

    K, slots = _schedule(counts)
    G, T = len(K), sum(K)
    off = np.concatenate([[0], np.cumsum(K)]).astype(np.int64)

    key = (tuple(K), MODE)
    if key not in _program_cache:
        _program_cache[key] = _build_program(K, MODE)
    nc = _program_cache[key]

    npdt = ml_dtypes.bfloat16 if MODE == "bf16" else np.float32

    # per-expert transposed weights [P, KT, OUT]: wT[p, k, o] = weight[e][o, 128k+p]
    wT = {}
    for e in set(e for s in slots for (e, _, n) in s if n > 0):
        wT[e] = np.ascontiguousarray(
            weight[e].T.reshape(KT, P, OUT_FEAT).transpose(1, 0, 2)).astype(npdt)

    in_maps = []
    for core in range(NCORES):
        xt = np.zeros((P, KT, T * P), dtype=npdt)
        wt = np.zeros((G, P, KT, OUT_FEAT), dtype=npdt)
        brep = np.zeros((G, 1, OUT_FEAT), dtype=np.float32)
        for g, (e, r0, n) in enumerate(slots[core]):
            if n > 0:
                blk = inp[r0:r0 + n].T.reshape(KT, P, n).transpose(1, 0, 2)
                col0 = int(off[g]) * P
                xt[:, :, col0:col0 + n] = blk.astype(npdt)
                wt[g] = wT[e]
                brep[g, 0] = bias[e]
        in_maps.append({"xt": xt, "wt": wt, "bias": brep})

    global LAST_RESULT
    res = run_bass_kernel_spmd(
        nc, in_maps, list(range(NCORES)),
        trace=TRACE, trace_cores=list(range(NCORES)) if TRACE else None,
        stitch_traces=False)
    LAST_RESULT = res

    out = np.empty((int(np.sum(np.asarray(counts, dtype=np.int64))), OUT_FEAT),
                   dtype=np.float32)
    for core in range(NCORES):
        arr = res.results[core]["out"]  # [P, T, OUT]
        for g, (e, r0, n) in enumerate(slots[core]):
            if n > 0:
                o0 = int(off[g])
                kg = K[g]
                blk = np.asarray(arr[:, o0:o0 + kg, :], dtype=np.float32)
                blk = blk.transpose(1, 0, 2).reshape(kg * P, OUT_FEAT)
                out[r0:r0 + n] = blk[:n]
    return out
